# revision 30
# baseline (speedup 1.0000x reference)
"""Bahdanau attention kernel for 8 TRN2 NeuronCores.

Reference math (per batch b):
    pq = q @ W_s                          (T, H)
    pe = enc @ W_h                        (S, H)
    score[t,s] = sum_h v[h] * tanh(pq[t,h] + pe[s,h])
    align = softmax_s(score masked by src_len)
    ctx = align @ enc                     (T, H)
    out = tanh([ctx, q] @ W_out)          (T, H)

Sharding: data-parallel over (b, T-half) -> 8 cores, 64 t's per core.

Score kernel: tanh(a+b) = G(u,w) with u=tanh(g*pq), w=tanh(g*pe)
(g=0.75), G approximated by a sparse bivariate polynomial
sum_p c_p u^{j_p} w^{m_p}, each (j,m) pair a K=512 block of PE matmuls:
score^T[s,t] += (w^m)[k,s]^T @ (c_p v u^j)[k,t].

Schedule (v2):
 - pq runs before peT on PE so the u-side planes are ready early; the
   per-kc w-chains (DVE low / Pool high) start right behind each peT
   tanh.
 - score matmuls are emitted kc-major so early-kc scores retire while
   late-kc w-planes are still being built; kc=3 is s-block-staggered
   with exp chasing.
 - ctx-first tail: PT[h,t] = sum_s enc[s,h] exp[s,t] accumulates in
   PSUM behind each exp; the PSUM->SBUF copy of PT multiplies in the
   softmax reciprocal (free normalization); out1T[o,t] accumulates
   onto an ap2T[o,t] = W_out[H:]^T q psum computed early, so the final
   tanh reads PSUM directly (no stt stage).  Output leaves transposed
   (outT [H, TC]); the host gather transposes back.
"""

import sys
from contextlib import ExitStack

import numpy as np

for _p in ("/opt/trn_rl_repo",):
    if _p not in sys.path:
        sys.path.insert(0, _p)

import ml_dtypes
import concourse.bacc as bacc
import concourse.tile as tile
from concourse import mybir
from concourse.bass_utils import run_bass_kernel_spmd

B, T, S, H = 4, 128, 512, 512
NCORES = 8
TC = 64          # t's per core
F32 = mybir.dt.float32
F32R = mybir.dt.float32r
BF16 = mybir.dt.bfloat16
AF = mybir.ActivationFunctionType
ALU = mybir.AluOpType
MASK_NEG = -1.0e30
BF16NP = np.dtype(ml_dtypes.bfloat16)

GAMMA = 0.75
# (j, m) monomial pairs in (u, w) and coefficients, greedy-fitted offline
PAIRS = [(0, 1), (0, 3), (0, 5), (0, 7), (1, 6), (6, 1), (11, 8), (10, 7),
         (1, 2), (2, 1), (2, 7), (9, 2), (3, 2)]
COEFS = [1.19785561, 0.13228916, 0.01488932, -0.44828153, -0.22014188,
         -0.08772471, -1.28001219, -1.33588108, -1.42375283, -1.30257206,
         1.9582073, 1.39007318, 0.72229679]
# u-power chain: u^a = u^b * u^c  (b, c already materialized)
U_CHAIN = [(2, 1, 1), (3, 2, 1), (6, 3, 3), (9, 6, 3), (10, 9, 1), (11, 9, 2)]
W_POWERS = sorted(set(m for _, m in PAIRS))          # [1,2,3,5,6,7,8]
U_POWERS = sorted(set(j for j, _ in PAIRS) - {0})
BIAS = [i for i, (j, _) in enumerate(PAIRS) if j == 0]
# m-sorted so early score matmuls only need the cheap low w-powers
NONBIAS = sorted((i for i, (j, _) in enumerate(PAIRS) if j != 0),
                 key=lambda p: (PAIRS[p][1], PAIRS[p][0]))


def _build_kernel(ctx: ExitStack, tc_: tile.TileContext, io: dict):
    nc = tc_.nc

    st = ctx.enter_context(tc_.tile_pool(name="statics", bufs=1))
    ps_score = ctx.enter_context(tc_.tile_pool(name="ps_score", bufs=1, space="PSUM"))
    ps_big = ctx.enter_context(tc_.tile_pool(name="ps_big", bufs=3, space="PSUM"))
    ps_small = ctx.enter_context(tc_.tile_pool(name="ps_small", bufs=1, space="PSUM"))
    ps_pt = ctx.enter_context(tc_.tile_pool(name="ps_pt", bufs=1, space="PSUM"))
    ps_o1 = ctx.enter_context(tc_.tile_pool(name="ps_o1", bufs=1, space="PSUM"))

    # ---- static tiles (host-packed) ----
    pew = st.tile([128, 4 * 1024], BF16, tag="pew")    # per hc: [encT 512 | wh 512]
    qs = st.tile([128, 4 * 576], BF16, tag="qs")       # per hc: [ws 512 | qT 64]
    wtop = st.tile([128, 4 * 512], BF16, tag="wtop")   # W_out[:H]: h-part, o-free
    wbot = st.tile([128, 4 * 512], F32R, tag="wbot")   # W_out[H:]: r-part, o-free
    encn = st.tile([128, 4 * 512], BF16, tag="encn")   # enc natural: [s-part, sb*512+h]
    vrep = st.tile([128, 256], BF16, tag="vrep")
    qT32 = st.tile([128, 4 * 64], F32R, tag="qT32")
    maskpack = st.tile([4, 384], BF16, tag="maskpack")  # [mneg 128 | bones 256]

    def encT(hc):
        return pew[:, hc * 1024: hc * 1024 + 512]

    def wh_blk(hc, kc):
        return pew[:, hc * 1024 + 512 + kc * 128: hc * 1024 + 512 + (kc + 1) * 128]

    def ws_blk(hc, kc):
        return qs[:, hc * 576 + kc * 128: hc * 576 + (kc + 1) * 128]

    def qT_bf(hc):
        return qs[:, hc * 576 + 512: hc * 576 + 576]

    # warmup scratch (no DMA deps): keeps the PE p-state ramp running
    warm_a = st.tile([128, 512], BF16, tag="warm_a")
    nc.vector.memset(warm_a[:], 0.001)
    ones128 = st.tile([128, 128], BF16, tag="ones128")
    nc.vector.memset(ones128[:], 1.0)

    # ---- DMA rings (issue cost ~500-1600ns on the issuing engine).
    # pew + qs are the critical early chunks; qT32 early for the ap2T
    # PE-filler; encn/wtop are needed only by the tail and go last on
    # SP.  ACT issues only the two qs chunks (its queue is the w1-tanh
    # critical path). ----
    nc.sync.dma_start(pew[:, 0:1024], io["pew"][0:128, :])
    nc.sync.dma_start(pew[:, 2 * 1024:3 * 1024], io["pew"][2 * 128:3 * 128, :])
    nc.sync.dma_start(qs[:, 576:1152], io["qs"][128:256, :])
    nc.sync.dma_start(qT32[:].rearrange("p (c t) -> p c t", c=4),
                      io["qT32"][:].rearrange("(c p) t -> p c t", c=4))
    for c in range(4):
        nc.sync.dma_start(wbot[:, c * 512:(c + 1) * 512], io["wbot"][c * 128:(c + 1) * 128, :])
    nc.sync.dma_start(encn[:].rearrange("p (sb h) -> p sb h", sb=4),
                      io["encn"][:].rearrange("(sb p) h -> p sb h", sb=4))
    nc.sync.dma_start(wtop[:, 0:1024].rearrange("p (c h) -> p c h", c=2),
                      io["wtop"][0:256, :].rearrange("(c p) h -> p c h", c=2))
    nc.sync.dma_start(wtop[:, 1024:2048].rearrange("p (c h) -> p c h", c=2),
                      io["wtop"][256:512, :].rearrange("(c p) h -> p c h", c=2))
    # Pool ring
    nc.gpsimd.dma_start(pew[:, 1024:2048], io["pew"][128:256, :])
    nc.gpsimd.dma_start(pew[:, 3 * 1024:4 * 1024], io["pew"][3 * 128:4 * 128, :])
    nc.gpsimd.dma_start(qs[:, 2 * 576:3 * 576], io["qs"][2 * 128:3 * 128, :])
    nc.gpsimd.dma_start(maskpack[:], io["maskpack"][:])
    nc.gpsimd.dma_start(vrep[:], io["vrep"][:])
    # ACT ring
    nc.scalar.dma_start(qs[:, 0:576], io["qs"][0:128, :])
    nc.scalar.dma_start(qs[:, 3 * 576:4 * 576], io["qs"][3 * 128:4 * 128, :])

    # ---- PE warmup (p-state ramp) ----
    wp = ps_big.tile([128, 512], F32, tag="big")
    for _ in range(4):
        nc.tensor.matmul(wp[:], warm_a[:, :128], warm_a[:], start=True, stop=True,
                         skip_group_check=True)
    nc.vector.tensor_copy(warm_a[0:1, 0:1], wp[0:1, 0:1])

    # scT openers (masked rows -1e30) are emitted later, right before the
    # first score block, so the maskpack DMA never stalls the PE queue
    scT_ev = ps_score.tile([128, 2 * 64], F32, tag="scT_ev")
    scT_od = ps_score.tile([128, 2 * 64], F32, tag="scT_od")

    def scT(sb):
        t_ = scT_ev if sb % 2 == 0 else scT_od
        return t_[:, (sb // 2) * 64:(sb // 2 + 1) * 64]

    def openers():
        nc.tensor.matmul(scT_ev[:], maskpack[:, 0:128],
                         maskpack[:, 128:384].rearrange("p (sb t) -> p sb t", sb=4)[:, 0::2, :],
                         start=True, stop=False, skip_group_check=True)
        nc.tensor.matmul(scT_od[:], maskpack[:, 0:128],
                         maskpack[:, 128:384].rearrange("p (sb t) -> p sb t", sb=4)[:, 1::2, :],
                         start=True, stop=False, skip_group_check=True)

    # ---- pq first: u-side tanh + chains become available early.  One
    # [128, 256] psum tile, a single u1 tanh after all four kc. ----
    u1 = st.tile([128, 4 * 64], BF16, tag="u1")
    pqp = ps_small.tile([128, 256], F32, tag="small")

    def pq_kc(kc):
        for hc in range(4):
            nc.tensor.matmul(pqp[:, kc * 64:(kc + 1) * 64], ws_blk(hc, kc),
                             qT_bf(hc), start=(kc == 0 and hc == 0),
                             stop=(kc == 3 and hc == 3), skip_group_check=True)
        if kc == 3:
            nc.scalar.activation(u1[:], pqp[:], AF.Tanh, scale=GAMMA)

    # ---- peT projection -> w = tanh(g*pe) per kc ----
    w_pl = {m: st.tile([128, 4 * 512], BF16, name=f"w{m}", tag=f"w{m}")
            for m in W_POWERS}

    def pe_kc(kc):
        pp = ps_big.tile([128, 512], F32, tag="big")
        for i, hc in enumerate((0, 1, 2, 3)):
            nc.tensor.matmul(pp[:], wh_blk(hc, kc), encT(hc),
                             start=(i == 0), stop=(i == 3))
        sl = slice(kc * 512, (kc + 1) * 512)
        nc.scalar.activation(w_pl[1][:, sl], pp[:], AF.Tanh, scale=GAMMA)

    def w_chain_low(kc):
        sl = slice(kc * 512, (kc + 1) * 512)
        nc.vector.tensor_tensor(w_pl[2][:, sl], w_pl[1][:, sl], w_pl[1][:, sl], op=ALU.mult)
        nc.vector.tensor_tensor(w_pl[3][:, sl], w_pl[1][:, sl], w_pl[2][:, sl], op=ALU.mult)

    def w_chain_high(kc):
        sl = slice(kc * 512, (kc + 1) * 512)
        nc.gpsimd.tensor_tensor(w_pl[6][:, sl], w_pl[3][:, sl], w_pl[3][:, sl], op=ALU.mult)
        nc.gpsimd.tensor_tensor(w_pl[7][:, sl], w_pl[1][:, sl], w_pl[6][:, sl], op=ALU.mult)
        nc.gpsimd.tensor_tensor(w_pl[8][:, sl], w_pl[2][:, sl], w_pl[6][:, sl], op=ALU.mult)

    def w_chain_w5(kc):
        sl = slice(kc * 512, (kc + 1) * 512)
        nc.vector.tensor_tensor(w_pl[5][:, sl], w_pl[2][:, sl], w_pl[3][:, sl], op=ALU.mult)

    # PE: pe0, pe1, pq 0..3, pe2, pe3, then the ap2T filler; the per-kc
    # w-chains (DVE lows+w5, Pool highs) chase each pe tanh, u/y/p planes
    # fill the DVE gaps
    pe_kc(0)
    w_chain_low(0)
    w_chain_w5(0)
    w_chain_high(0)
    pe_kc(1)
    w_chain_low(1)
    w_chain_w5(1)
    w_chain_high(1)
    pq_kc(0)
    pq_kc(1)
    pq_kc(2)
    pq_kc(3)
    pe_kc(2)
    w_chain_low(2)
    w_chain_w5(2)
    w_chain_high(2)
    pe_kc(3)
    w_chain_low(3)
    w_chain_w5(3)
    w_chain_high(3)

    # u-side chains + y/p planes on DVE/ACT (u1 ready after pq3)
    u_pl = {1: u1}
    for a, b_, c_ in U_CHAIN:
        u_pl[a] = st.tile([128, 256], BF16, name=f"u{a}", tag=f"u{a}")
        nc.vector.tensor_tensor(u_pl[a][:], u_pl[b_][:], u_pl[c_][:], op=ALU.mult)
    y_pl = {0: vrep}
    for i, j in enumerate(U_POWERS):
        y_pl[j] = st.tile([128, 256], BF16, name=f"y{j}", tag=f"y{j}")
        nc.vector.tensor_tensor(y_pl[j][:], u_pl[j][:], vrep, op=ALU.mult)
    # p-planes in score-consumption order (NONBIAS is m-sorted); bias
    # p-planes only need one column per kc -> tiny 4-col ops
    p_pl = {}
    for i, p in enumerate(NONBIAS):
        (j, m), c_ = PAIRS[p], COEFS[p]
        pt = st.tile([128, 256], BF16, name=f"p{j}_{m}", tag=f"p{j}_{m}")
        if i % 2 == 0:
            nc.vector.tensor_scalar(pt[:], y_pl[j][:], float(c_), None, op0=ALU.mult)
        else:
            nc.scalar.activation(pt[:], y_pl[j][:], AF.Copy, scale=float(c_))
        p_pl[p] = pt
    for p in BIAS:
        (j, m), c_ = PAIRS[p], COEFS[p]
        pt = st.tile([128, 4], BF16, name=f"pb{m}", tag=f"pb{m}")
        nc.vector.tensor_scalar(pt[:], vrep[:, 0:256:64], float(c_), None,
                                op0=ALU.mult)
        p_pl[p] = pt

    # ---- ap2T[o, t] = W_out[H:]^T @ q accumulated into the o1 psum;
    # out1T later accumulates on top, so the final tanh reads PSUM.
    # rc-outer so the late-landing wbot chunks stall only the tail ----
    o1T = ps_o1.tile([128, 4 * 64], F32, tag="o1")
    for rc in range(4):
        for oc in range(4):
            nc.tensor.matmul(o1T[:, oc * 64:(oc + 1) * 64],
                             wbot[:, rc * 512 + oc * 128: rc * 512 + (oc + 1) * 128],
                             qT32[:, rc * 64:(rc + 1) * 64],
                             start=(rc == 0 and oc == 0), stop=False,
                             skip_group_check=True)

    # ---- score matmuls, kc-major; fb bias rides along ----
    fb = ps_small.tile([128, 4], F32, tag="small")

    def score_kc(kc, stagger=False):
        # bias pairs (N=1 matmuls into the fb column psum)
        for sb in range(4):
            for i, p in enumerate(BIAS):
                j, m = PAIRS[p]
                nc.tensor.matmul(
                    fb[:, sb:sb + 1],
                    w_pl[m][:, kc * 512 + sb * 128: kc * 512 + (sb + 1) * 128],
                    p_pl[p][:, kc:kc + 1],
                    start=(kc == 0 and sb == 0 and i == 0),
                    stop=(kc == 3 and sb == 3 and i == len(BIAS) - 1),
                    skip_group_check=True)
        if kc == 3:
            nc.vector.tensor_copy(fbs[:], fb[:])
        for sb in range(4):
            for i, p in enumerate(NONBIAS):
                j, m = PAIRS[p]
                last = (kc == 3 and i == len(NONBIAS) - 1 and sb >= 2)
                nc.tensor.matmul(
                    scT(sb),
                    w_pl[m][:, kc * 512 + sb * 128: kc * 512 + (sb + 1) * 128],
                    p_pl[p][:, kc * 64:(kc + 1) * 64],
                    start=False, stop=last, skip_group_check=True)
            if stagger:
                exp_sb(sb)
                pt_sb(sb)

    expT = st.tile([128, 4 * 64], BF16, tag="expT")
    outT_sb = st.tile([128, 4 * 64], F32, tag="outT")
    rTrep = st.tile([128, 64], F32, tag="rTrep")
    PTs = st.tile([128, 256], BF16, tag="PTs")
    PT = ps_pt.tile([128, 256], F32, tag="pt")
    denrep = ps_small.tile([128, 64], F32, tag="small")
    fbs = st.tile([128, 4], F32, tag="fbs")

    def exp_sb(sb):
        nc.scalar.activation(expT[:, sb * 64:(sb + 1) * 64],
                             scT(sb), AF.Exp,
                             bias=fbs[:, sb:sb + 1])

    def pt_sb(sb):
        # PT[h, t] += enc_nat[sb][s, h]^T-contract exp[sb][s, t]
        # (start only on the very first matmul: start marks the whole 2KB
        # PSUM zero-region, a second start=True would wipe prior chunks)
        for hc in range(4):
            nc.tensor.matmul(PT[:, hc * 64:(hc + 1) * 64],
                             encn[:, sb * 512 + hc * 128: sb * 512 + (hc + 1) * 128],
                             expT[:, sb * 64:(sb + 1) * 64],
                             start=(sb == 0 and hc == 0),
                             stop=(sb == 3 and hc == 3),
                             skip_group_check=True)
        # denominator, replicated across all 128 partitions (all-ones
        # stationary): denrep[p, t] = sum_s exp[s, t] for every p
        nc.tensor.matmul(denrep[:], ones128[:],
                         expT[:, sb * 64:(sb + 1) * 64],
                         start=(sb == 0), stop=(sb == 3),
                         skip_group_check=True)

    openers()
    score_kc(0)
    score_kc(1)
    score_kc(2)
    score_kc(3, stagger=True)

    # ---- tail: normalize PT during the PSUM->SBUF copy, then
    # out1T[o,t] += wtop[h,o]^T PTs_n[h,t] on top of ap2T, tanh straight
    # from PSUM, transposed output DMA ----
    nc.vector.reciprocal(rTrep[:], denrep[:])
    for hc in range(4):
        nc.vector.tensor_tensor(PTs[:, hc * 64:(hc + 1) * 64],
                                PT[:, hc * 64:(hc + 1) * 64],
                                rTrep[:], op=ALU.mult)
    # single stop on the very last matmul: per-oc stops made later oc
    # matmuls serialize against the ACT tanh reads of the same bank
    for oc in range(4):
        for hc in range(4):
            nc.tensor.matmul(o1T[:, oc * 64:(oc + 1) * 64],
                             wtop[:, hc * 512 + oc * 128: hc * 512 + (oc + 1) * 128],
                             PTs[:, hc * 64:(hc + 1) * 64],
                             start=False, stop=(oc == 3 and hc == 3),
                             skip_group_check=True)
    for oc in range(4):
        nc.scalar.activation(outT_sb[:, oc * 64:(oc + 1) * 64],
                             o1T[:, oc * 64:(oc + 1) * 64], AF.Tanh)
        if oc % 2 == 0:
            nc.sync.dma_start(io["outT"][oc * 128:(oc + 1) * 128, :],
                              outT_sb[:, oc * 64:(oc + 1) * 64])
        else:
            nc.gpsimd.dma_start(io["outT"][oc * 128:(oc + 1) * 128, :],
                                outT_sb[:, oc * 64:(oc + 1) * 64])


_NC_CACHE = None


def _get_nc():
    global _NC_CACHE
    if _NC_CACHE is None:
        nc = bacc.Bacc("TRN2", target_bir_lowering=False, debug=False,
                       num_devices=NCORES)
        io = {
            "pew": nc.dram_tensor("pew", [H, 2 * H], BF16, kind="ExternalInput").ap(),
            "qs": nc.dram_tensor("qs", [H, H + TC], BF16, kind="ExternalInput").ap(),
            "wtop": nc.dram_tensor("wtop", [H, H], BF16, kind="ExternalInput").ap(),
            "wbot": nc.dram_tensor("wbot", [H, H], F32R, kind="ExternalInput").ap(),
            "encn": nc.dram_tensor("encn", [S, H], BF16, kind="ExternalInput").ap(),
            "qT32": nc.dram_tensor("qT32", [H, TC], F32R, kind="ExternalInput").ap(),
            "vrep": nc.dram_tensor("vrep", [128, 256], BF16, kind="ExternalInput").ap(),
            "maskpack": nc.dram_tensor("maskpack", [4, 384], BF16, kind="ExternalInput").ap(),
            "outT": nc.dram_tensor("outT", [H, TC], F32, kind="ExternalOutput").ap(),
        }
        with tile.TileContext(nc) as tc_:
            with ExitStack() as ctx:
                _build_kernel(ctx, tc_, io)
        nc.compile()
        _NC_CACHE = nc
    return _NC_CACHE


def _make_in_maps(query, encoder_outputs, src_lengths, W_h, W_s, v, W_out):
    f = lambda a: np.ascontiguousarray(np.asarray(a, dtype=np.float32))
    query, encoder_outputs = f(query), f(encoder_outputs)
    W_h, W_s, v, W_out = f(W_h), f(W_s), f(v), f(W_out)
    lens = np.asarray(src_lengths)
    bf = lambda a: np.ascontiguousarray(np.asarray(a).astype(BF16NP))
    s_iota = np.arange(S)
    bones = np.kron(np.eye(4), np.ones((1, 64))).astype(np.float32)   # (4, 256)
    v4 = v.reshape(4, 128).T                                          # v4[k, kc]
    vrep = np.repeat(v4, 64, axis=1)                                  # [128, 4*64]
    wh_bf, ws_bf = bf(W_h), bf(W_s)
    wtop_bf = bf(W_out[:H])
    wbot32 = np.ascontiguousarray(W_out[H:])
    in_maps = []
    for j in range(NCORES):
        b, half = j // 2, j % 2
        mg = np.where(s_iota < int(lens[b]), 0.0, MASK_NEG).astype(np.float32)
        qT = np.ascontiguousarray(query[b, half * TC:(half + 1) * TC, :].T)
        in_maps.append({
            "pew": bf(np.concatenate([encoder_outputs[b].T, W_h], axis=1)),
            "qs": bf(np.concatenate([W_s, qT], axis=1)),
            "wtop": wtop_bf, "wbot": wbot32, "qT32": qT,
            "encn": bf(encoder_outputs[b]),
            "vrep": bf(vrep[:, :256]),
            "maskpack": bf(np.concatenate([mg.reshape(4, 128), bones], axis=1)),
        })
    return in_maps


def kernel(query, encoder_outputs, src_lengths, W_h, W_s, v, W_out, _trace=False):
    nc = _get_nc()
    in_maps = _make_in_maps(query, encoder_outputs, src_lengths, W_h, W_s, v, W_out)
    res = run_bass_kernel_spmd(nc, in_maps, list(range(NCORES)), trace=_trace)
    out = np.empty((B, T, H), dtype=np.float32)
    for j in range(NCORES):
        b, half = j // 2, j % 2
        out[b, half * TC:(half + 1) * TC, :] = res.results[j]["outT"].T
    if _trace:
        return out, res
    return out


# revision 33
# speedup vs baseline: 1.0679x; 1.0679x over previous
"""Bahdanau attention kernel for 8 TRN2 NeuronCores.

Reference math (per batch b):
    pq = q @ W_s                          (T, H)
    pe = enc @ W_h                        (S, H)
    score[t,s] = sum_h v[h] * tanh(pq[t,h] + pe[s,h])
    align = softmax_s(score masked by src_len)
    ctx = align @ enc                     (T, H)
    out = tanh([ctx, q] @ W_out)          (T, H)

Sharding: data-parallel over (b, T-half) -> 8 cores, 64 t's per core.

Score kernel: tanh(a+b) = G(u,w) with u=tanh(g*pq), w=tanh(g*pe)
(g=0.75), G approximated by a sparse bivariate polynomial
sum_p c_p u^{j_p} w^{m_p}, each (j,m) pair a K=512 block of PE matmuls:
score^T[s,t] += (w^m)[k,s]^T @ (c_p v u^j)[k,t].

Schedule (v2):
 - pq runs before peT on PE so the u-side planes are ready early; the
   per-kc w-chains (DVE low / Pool high) start right behind each peT
   tanh.
 - score matmuls are emitted kc-major so early-kc scores retire while
   late-kc w-planes are still being built; kc=3 is s-block-staggered
   with exp chasing.
 - ctx-first tail: PT[h,t] = sum_s enc[s,h] exp[s,t] accumulates in
   PSUM behind each exp; the PSUM->SBUF copy of PT multiplies in the
   softmax reciprocal (free normalization); out1T[o,t] accumulates
   onto an ap2T[o,t] = W_out[H:]^T q psum computed early, so the final
   tanh reads PSUM directly (no stt stage).  Output leaves transposed
   (outT [H, TC]); the host gather transposes back.
"""

import sys
from contextlib import ExitStack

import numpy as np

for _p in ("/opt/trn_rl_repo",):
    if _p not in sys.path:
        sys.path.insert(0, _p)

import ml_dtypes
import concourse.bacc as bacc
import concourse.tile as tile
from concourse import mybir
from concourse.bass_utils import run_bass_kernel_spmd

B, T, S, H = 4, 128, 512, 512
NCORES = 8
TC = 64          # t's per core
F32 = mybir.dt.float32
F32R = mybir.dt.float32r
BF16 = mybir.dt.bfloat16
AF = mybir.ActivationFunctionType
ALU = mybir.AluOpType
MASK_NEG = -1.0e30
BF16NP = np.dtype(ml_dtypes.bfloat16)

GAMMA = 0.75
# (j, m) monomial pairs in (u, w) and coefficients, greedy-fitted offline
PAIRS = [(0, 1), (0, 3), (0, 5), (0, 7), (1, 6), (6, 1), (11, 8), (10, 7),
         (1, 2), (2, 1), (2, 7), (9, 2), (3, 2)]
COEFS = [1.19785561, 0.13228916, 0.01488932, -0.44828153, -0.22014188,
         -0.08772471, -1.28001219, -1.33588108, -1.42375283, -1.30257206,
         1.9582073, 1.39007318, 0.72229679]
# u-power chain: u^a = u^b * u^c  (b, c already materialized)
U_CHAIN = [(2, 1, 1), (3, 2, 1), (6, 3, 3), (9, 6, 3), (10, 9, 1), (11, 9, 2)]
W_POWERS = sorted(set(m for _, m in PAIRS))          # [1,2,3,5,6,7,8]
U_POWERS = sorted(set(j for j, _ in PAIRS) - {0})
BIAS = [i for i, (j, _) in enumerate(PAIRS) if j == 0]
# m-sorted so early score matmuls only need the cheap low w-powers
NONBIAS = sorted((i for i, (j, _) in enumerate(PAIRS) if j != 0),
                 key=lambda p: (PAIRS[p][1], PAIRS[p][0]))


def _build_kernel(ctx: ExitStack, tc_: tile.TileContext, io: dict):
    nc = tc_.nc

    st = ctx.enter_context(tc_.tile_pool(name="statics", bufs=1))
    ps_score = ctx.enter_context(tc_.tile_pool(name="ps_score", bufs=1, space="PSUM"))
    ps_big = ctx.enter_context(tc_.tile_pool(name="ps_big", bufs=3, space="PSUM"))
    ps_small = ctx.enter_context(tc_.tile_pool(name="ps_small", bufs=1, space="PSUM"))
    ps_pt = ctx.enter_context(tc_.tile_pool(name="ps_pt", bufs=1, space="PSUM"))
    ps_o1 = ctx.enter_context(tc_.tile_pool(name="ps_o1", bufs=1, space="PSUM"))

    # ---- static tiles (host-packed) ----
    pew = st.tile([128, 4 * 1024], BF16, tag="pew")    # per hc: [encT 512 | wh 512]
    qs = st.tile([128, 4 * 576], BF16, tag="qs")       # per hc: [ws 512 | qT 64]
    wtop = st.tile([128, 4 * 512], BF16, tag="wtop")   # W_out[:H]: h-part, o-free
    wbot = st.tile([128, 4 * 512], F32R, tag="wbot")   # W_out[H:]: r-part, o-free
    encn = st.tile([128, 4 * 512], BF16, tag="encn")   # enc natural: [s-part, sb*512+h]
    vrep = st.tile([128, 256], BF16, tag="vrep")
    qT32 = st.tile([128, 4 * 64], F32R, tag="qT32")
    maskpack = st.tile([4, 384], BF16, tag="maskpack")  # [mneg 128 | bones 256]

    def encT(hc):
        return pew[:, hc * 1024: hc * 1024 + 512]

    def wh_blk(hc, kc):
        return pew[:, hc * 1024 + 512 + kc * 128: hc * 1024 + 512 + (kc + 1) * 128]

    def ws_blk(hc, kc):
        return qs[:, hc * 576 + kc * 128: hc * 576 + (kc + 1) * 128]

    def qT_bf(hc):
        return qs[:, hc * 576 + 512: hc * 576 + 576]

    # warmup scratch (no DMA deps): keeps the PE p-state ramp running
    warm_a = st.tile([128, 512], BF16, tag="warm_a")
    nc.vector.memset(warm_a[:], 0.001)
    ones128 = st.tile([128, 128], BF16, tag="ones128")
    nc.vector.memset(ones128[:], 1.0)

    # ---- DMA rings (issue cost ~500-1600ns on the issuing engine).
    # pew + qs are the critical early chunks; qT32 early for the ap2T
    # PE-filler; encn/wtop are needed only by the tail and go last on
    # SP.  ACT issues only the two qs chunks (its queue is the w1-tanh
    # critical path). ----
    nc.sync.dma_start(pew[:, 0:1024], io["pew"][0:128, :])
    nc.sync.dma_start(pew[:, 2 * 1024:3 * 1024], io["pew"][2 * 128:3 * 128, :])
    nc.sync.dma_start(qs[:, 576:1152], io["qs"][128:256, :])
    nc.sync.dma_start(qT32[:].rearrange("p (c t) -> p c t", c=4),
                      io["qT32"][:].rearrange("(c p) t -> p c t", c=4))
    for c in range(4):
        nc.sync.dma_start(wbot[:, c * 512:(c + 1) * 512], io["wbot"][c * 128:(c + 1) * 128, :])
    nc.sync.dma_start(encn[:].rearrange("p (sb h) -> p sb h", sb=4),
                      io["encn"][:].rearrange("(sb p) h -> p sb h", sb=4))
    nc.sync.dma_start(wtop[:, 0:1024].rearrange("p (c h) -> p c h", c=2),
                      io["wtop"][0:256, :].rearrange("(c p) h -> p c h", c=2))
    nc.sync.dma_start(wtop[:, 1024:2048].rearrange("p (c h) -> p c h", c=2),
                      io["wtop"][256:512, :].rearrange("(c p) h -> p c h", c=2))
    # Pool ring
    nc.gpsimd.dma_start(pew[:, 1024:2048], io["pew"][128:256, :])
    nc.gpsimd.dma_start(pew[:, 3 * 1024:4 * 1024], io["pew"][3 * 128:4 * 128, :])
    nc.gpsimd.dma_start(qs[:, 2 * 576:3 * 576], io["qs"][2 * 128:3 * 128, :])
    nc.gpsimd.dma_start(maskpack[:], io["maskpack"][:])
    nc.gpsimd.dma_start(vrep[:], io["vrep"][:])
    # ACT ring
    nc.scalar.dma_start(qs[:, 0:576], io["qs"][0:128, :])
    nc.scalar.dma_start(qs[:, 3 * 576:4 * 576], io["qs"][3 * 128:4 * 128, :])

    # ---- PE warmup (p-state ramp) ----
    wp = ps_big.tile([128, 512], F32, tag="big")
    for _ in range(4):
        nc.tensor.matmul(wp[:], warm_a[:, :128], warm_a[:], start=True, stop=True,
                         skip_group_check=True)
    nc.vector.tensor_copy(warm_a[0:1, 0:1], wp[0:1, 0:1])

    # scT openers (masked rows -1e30) are emitted later, right before the
    # first score block, so the maskpack DMA never stalls the PE queue
    scT_ev = ps_score.tile([128, 2 * 64], F32, tag="scT_ev")
    scT_od = ps_score.tile([128, 2 * 64], F32, tag="scT_od")

    def scT(sb):
        t_ = scT_ev if sb % 2 == 0 else scT_od
        return t_[:, (sb // 2) * 64:(sb // 2 + 1) * 64]

    def openers():
        nc.tensor.matmul(scT_ev[:], maskpack[:, 0:128],
                         maskpack[:, 128:384].rearrange("p (sb t) -> p sb t", sb=4)[:, 0::2, :],
                         start=True, stop=False, skip_group_check=True)
        nc.tensor.matmul(scT_od[:], maskpack[:, 0:128],
                         maskpack[:, 128:384].rearrange("p (sb t) -> p sb t", sb=4)[:, 1::2, :],
                         start=True, stop=False, skip_group_check=True)

    # ---- pq first: u-side tanh + chains become available early.  One
    # [128, 256] psum tile, a single u1 tanh after all four kc. ----
    u1 = st.tile([128, 4 * 64], BF16, tag="u1")
    pqp = ps_small.tile([128, 256], F32, tag="small")

    def pq_kc(kc):
        for hc in range(4):
            nc.tensor.matmul(pqp[:, kc * 64:(kc + 1) * 64], ws_blk(hc, kc),
                             qT_bf(hc), start=(kc == 0 and hc == 0),
                             stop=(kc == 3 and hc == 3), skip_group_check=True)
        if kc == 3:
            nc.scalar.activation(u1[:], pqp[:], AF.Tanh, scale=GAMMA)

    # ---- peT projection -> w = tanh(g*pe) per kc ----
    w_pl = {m: st.tile([128, 4 * 512], BF16, name=f"w{m}", tag=f"w{m}")
            for m in W_POWERS}

    def pe_kc(kc):
        pp = ps_big.tile([128, 512], F32, tag="big")
        for i, hc in enumerate((0, 1, 2, 3)):
            nc.tensor.matmul(pp[:], wh_blk(hc, kc), encT(hc),
                             start=(i == 0), stop=(i == 3))
        sl = slice(kc * 512, (kc + 1) * 512)
        nc.scalar.activation(w_pl[1][:, sl], pp[:], AF.Tanh, scale=GAMMA)

    def w_chain_low(kc):
        sl = slice(kc * 512, (kc + 1) * 512)
        nc.vector.tensor_tensor(w_pl[2][:, sl], w_pl[1][:, sl], w_pl[1][:, sl], op=ALU.mult)
        nc.vector.tensor_tensor(w_pl[3][:, sl], w_pl[1][:, sl], w_pl[2][:, sl], op=ALU.mult)

    def w_chain_high(kc):
        sl = slice(kc * 512, (kc + 1) * 512)
        nc.gpsimd.tensor_tensor(w_pl[6][:, sl], w_pl[3][:, sl], w_pl[3][:, sl], op=ALU.mult)
        nc.gpsimd.tensor_tensor(w_pl[7][:, sl], w_pl[1][:, sl], w_pl[6][:, sl], op=ALU.mult)
        nc.gpsimd.tensor_tensor(w_pl[8][:, sl], w_pl[2][:, sl], w_pl[6][:, sl], op=ALU.mult)

    def w_chain_w5(kc):
        sl = slice(kc * 512, (kc + 1) * 512)
        nc.vector.tensor_tensor(w_pl[5][:, sl], w_pl[2][:, sl], w_pl[3][:, sl], op=ALU.mult)

    # u-side tiles (filled below, interleaved with the w-chains)
    u_pl = {1: u1}
    for a, _, _ in U_CHAIN:
        u_pl[a] = st.tile([128, 256], BF16, name=f"u{a}", tag=f"u{a}")
    y_pl = {0: vrep}
    for j in U_POWERS:
        y_pl[j] = st.tile([128, 256], BF16, name=f"y{j}", tag=f"y{j}")
    p_pl = {}
    for p in NONBIAS:
        j, m = PAIRS[p]
        p_pl[p] = st.tile([128, 256], BF16, name=f"p{j}_{m}", tag=f"p{j}_{m}")
    for p in BIAS:
        j, m = PAIRS[p]
        p_pl[p] = st.tile([128, 4], BF16, name=f"pb{m}", tag=f"pb{m}")

    def u_chain_ops(lo, hi):
        for a, b_, c_ in U_CHAIN[lo:hi]:
            nc.vector.tensor_tensor(u_pl[a][:], u_pl[b_][:], u_pl[c_][:], op=ALU.mult)

    def y_ops(lo, hi):
        for j in U_POWERS[lo:hi]:
            nc.vector.tensor_tensor(y_pl[j][:], u_pl[j][:], vrep, op=ALU.mult)

    def p_ops(idxs):
        for i, p in idxs:
            (j, m), c_ = PAIRS[p], COEFS[p]
            if i % 2 == 0:
                nc.vector.tensor_scalar(p_pl[p][:], y_pl[j][:], float(c_),
                                        None, op0=ALU.mult)
            else:
                nc.scalar.activation(p_pl[p][:], y_pl[j][:], AF.Copy,
                                     scale=float(c_))

    # PE: pe0, pe1, pq 0..3, pe2, pe3, then the ap2T filler.  DVE weaves
    # the per-kc lows (which feed the Pool highs) with the u/y/p planes
    # (which gate the first score matmuls).
    pe_kc(0)
    w_chain_low(0)
    pe_kc(1)
    pq_kc(0)
    pq_kc(1)
    pq_kc(2)
    pq_kc(3)
    w_chain_high(0)
    u_chain_ops(0, 3)          # u2, u3, u6
    w_chain_low(1)
    pe_kc(2)
    w_chain_high(1)
    u_chain_ops(3, 6)          # u9, u10, u11
    w_chain_low(2)
    pe_kc(3)
    w_chain_high(2)
    y_ops(0, 4)
    w_chain_low(3)
    w_chain_high(3)
    y_ops(4, 7)
    for p in BIAS:
        (j, m), c_ = PAIRS[p], COEFS[p]
        nc.vector.tensor_scalar(p_pl[p][:], vrep[:, 0:256:64], float(c_),
                                None, op0=ALU.mult)
    p_ops(list(enumerate(NONBIAS)))
    w_chain_w5(0)
    w_chain_w5(1)
    w_chain_w5(2)
    w_chain_w5(3)

    # ---- ap2T[o, t] = W_out[H:]^T @ q accumulated into the o1 psum;
    # out1T later accumulates on top, so the final tanh reads PSUM.
    # rc-outer so the late-landing wbot chunks stall only the tail ----
    o1T = ps_o1.tile([128, 4 * 64], F32, tag="o1")
    for rc in range(4):
        for oc in range(4):
            nc.tensor.matmul(o1T[:, oc * 64:(oc + 1) * 64],
                             wbot[:, rc * 512 + oc * 128: rc * 512 + (oc + 1) * 128],
                             qT32[:, rc * 64:(rc + 1) * 64],
                             start=(rc == 0 and oc == 0), stop=False,
                             skip_group_check=True)

    # ---- score matmuls, kc-major; fb bias rides along ----
    fb = ps_small.tile([128, 4], F32, tag="small")

    def fb_all():
        # bias pairs: N=1 matmuls into the fb column psum, all kc at once
        # (gated by w5/w7 which are late anyway), then the fbs copy
        for kc in range(4):
            for sb in range(4):
                for i, p in enumerate(BIAS):
                    j, m = PAIRS[p]
                    nc.tensor.matmul(
                        fb[:, sb:sb + 1],
                        w_pl[m][:, kc * 512 + sb * 128: kc * 512 + (sb + 1) * 128],
                        p_pl[p][:, kc:kc + 1],
                        start=(kc == 0 and sb == 0 and i == 0),
                        stop=(kc == 3 and sb == 3 and i == len(BIAS) - 1),
                        skip_group_check=True)
        nc.vector.tensor_copy(fbs[:], fb[:])

    def score_kc(kc, stagger=False):
        for sb in range(4):
            for i, p in enumerate(NONBIAS):
                j, m = PAIRS[p]
                last = (kc == 3 and i == len(NONBIAS) - 1 and sb >= 2)
                nc.tensor.matmul(
                    scT(sb),
                    w_pl[m][:, kc * 512 + sb * 128: kc * 512 + (sb + 1) * 128],
                    p_pl[p][:, kc * 64:(kc + 1) * 64],
                    start=False, stop=last, skip_group_check=True)
            if stagger:
                exp_sb(sb)
                pt_sb(sb)

    expT = st.tile([128, 4 * 64], BF16, tag="expT")
    outT_sb = st.tile([128, 4 * 64], F32, tag="outT")
    rTrep = st.tile([128, 64], F32, tag="rTrep")
    PTs = st.tile([128, 256], BF16, tag="PTs")
    PT = ps_pt.tile([128, 256], F32, tag="pt")
    denrep = ps_small.tile([128, 64], F32, tag="small")
    fbs = st.tile([128, 4], F32, tag="fbs")

    def exp_sb(sb):
        nc.scalar.activation(expT[:, sb * 64:(sb + 1) * 64],
                             scT(sb), AF.Exp,
                             bias=fbs[:, sb:sb + 1])

    def pt_sb(sb):
        # PT[h, t] += enc_nat[sb][s, h]^T-contract exp[sb][s, t]
        # (start only on the very first matmul: start marks the whole 2KB
        # PSUM zero-region, a second start=True would wipe prior chunks)
        for hc in range(4):
            nc.tensor.matmul(PT[:, hc * 64:(hc + 1) * 64],
                             encn[:, sb * 512 + hc * 128: sb * 512 + (hc + 1) * 128],
                             expT[:, sb * 64:(sb + 1) * 64],
                             start=(sb == 0 and hc == 0),
                             stop=(sb == 3 and hc == 3),
                             skip_group_check=True)
        # denominator, replicated across all 128 partitions (all-ones
        # stationary): denrep[p, t] = sum_s exp[s, t] for every p
        nc.tensor.matmul(denrep[:], ones128[:],
                         expT[:, sb * 64:(sb + 1) * 64],
                         start=(sb == 0), stop=(sb == 3),
                         skip_group_check=True)

    openers()
    score_kc(0)
    score_kc(1)
    score_kc(2)
    fb_all()
    score_kc(3, stagger=True)

    # ---- tail: normalize PT during the PSUM->SBUF copy, then
    # out1T[o,t] += wtop[h,o]^T PTs_n[h,t] on top of ap2T, tanh straight
    # from PSUM, transposed output DMA ----
    nc.vector.reciprocal(rTrep[:], denrep[:])
    for hc in range(4):
        nc.vector.tensor_tensor(PTs[:, hc * 64:(hc + 1) * 64],
                                PT[:, hc * 64:(hc + 1) * 64],
                                rTrep[:], op=ALU.mult)
    # single stop on the very last matmul: per-oc stops made later oc
    # matmuls serialize against the ACT tanh reads of the same bank
    for oc in range(4):
        for hc in range(4):
            nc.tensor.matmul(o1T[:, oc * 64:(oc + 1) * 64],
                             wtop[:, hc * 512 + oc * 128: hc * 512 + (oc + 1) * 128],
                             PTs[:, hc * 64:(hc + 1) * 64],
                             start=False, stop=(oc == 3 and hc == 3),
                             skip_group_check=True)
    for oc in range(4):
        nc.scalar.activation(outT_sb[:, oc * 64:(oc + 1) * 64],
                             o1T[:, oc * 64:(oc + 1) * 64], AF.Tanh)
        if oc % 2 == 0:
            nc.sync.dma_start(io["outT"][oc * 128:(oc + 1) * 128, :],
                              outT_sb[:, oc * 64:(oc + 1) * 64])
        else:
            nc.gpsimd.dma_start(io["outT"][oc * 128:(oc + 1) * 128, :],
                                outT_sb[:, oc * 64:(oc + 1) * 64])


_NC_CACHE = None


def _get_nc():
    global _NC_CACHE
    if _NC_CACHE is None:
        nc = bacc.Bacc("TRN2", target_bir_lowering=False, debug=False,
                       num_devices=NCORES)
        io = {
            "pew": nc.dram_tensor("pew", [H, 2 * H], BF16, kind="ExternalInput").ap(),
            "qs": nc.dram_tensor("qs", [H, H + TC], BF16, kind="ExternalInput").ap(),
            "wtop": nc.dram_tensor("wtop", [H, H], BF16, kind="ExternalInput").ap(),
            "wbot": nc.dram_tensor("wbot", [H, H], F32R, kind="ExternalInput").ap(),
            "encn": nc.dram_tensor("encn", [S, H], BF16, kind="ExternalInput").ap(),
            "qT32": nc.dram_tensor("qT32", [H, TC], F32R, kind="ExternalInput").ap(),
            "vrep": nc.dram_tensor("vrep", [128, 256], BF16, kind="ExternalInput").ap(),
            "maskpack": nc.dram_tensor("maskpack", [4, 384], BF16, kind="ExternalInput").ap(),
            "outT": nc.dram_tensor("outT", [H, TC], F32, kind="ExternalOutput").ap(),
        }
        with tile.TileContext(nc) as tc_:
            with ExitStack() as ctx:
                _build_kernel(ctx, tc_, io)
        nc.compile()
        _NC_CACHE = nc
    return _NC_CACHE


def _make_in_maps(query, encoder_outputs, src_lengths, W_h, W_s, v, W_out):
    f = lambda a: np.ascontiguousarray(np.asarray(a, dtype=np.float32))
    query, encoder_outputs = f(query), f(encoder_outputs)
    W_h, W_s, v, W_out = f(W_h), f(W_s), f(v), f(W_out)
    lens = np.asarray(src_lengths)
    bf = lambda a: np.ascontiguousarray(np.asarray(a).astype(BF16NP))
    s_iota = np.arange(S)
    bones = np.kron(np.eye(4), np.ones((1, 64))).astype(np.float32)   # (4, 256)
    v4 = v.reshape(4, 128).T                                          # v4[k, kc]
    vrep = np.repeat(v4, 64, axis=1)                                  # [128, 4*64]
    wh_bf, ws_bf = bf(W_h), bf(W_s)
    wtop_bf = bf(W_out[:H])
    wbot32 = np.ascontiguousarray(W_out[H:])
    in_maps = []
    for j in range(NCORES):
        b, half = j // 2, j % 2
        mg = np.where(s_iota < int(lens[b]), 0.0, MASK_NEG).astype(np.float32)
        qT = np.ascontiguousarray(query[b, half * TC:(half + 1) * TC, :].T)
        in_maps.append({
            "pew": bf(np.concatenate([encoder_outputs[b].T, W_h], axis=1)),
            "qs": bf(np.concatenate([W_s, qT], axis=1)),
            "wtop": wtop_bf, "wbot": wbot32, "qT32": qT,
            "encn": bf(encoder_outputs[b]),
            "vrep": bf(vrep[:, :256]),
            "maskpack": bf(np.concatenate([mg.reshape(4, 128), bones], axis=1)),
        })
    return in_maps


def kernel(query, encoder_outputs, src_lengths, W_h, W_s, v, W_out, _trace=False):
    nc = _get_nc()
    in_maps = _make_in_maps(query, encoder_outputs, src_lengths, W_h, W_s, v, W_out)
    res = run_bass_kernel_spmd(nc, in_maps, list(range(NCORES)), trace=_trace)
    out = np.empty((B, T, H), dtype=np.float32)
    for j in range(NCORES):
        b, half = j // 2, j % 2
        out[b, half * TC:(half + 1) * TC, :] = res.results[j]["outT"].T
    if _trace:
        return out, res
    return out
